# revision 1
# baseline (speedup 1.0000x reference)
"""Trainium2 Bass kernel for DAV-block patch cosine similarity.

Math (equivalent to the reference's 9-shift loop, ~4.5x less work):
    S[y,x,d] = sum_c q[c,y,x]*K[c,y,x,d]      (channel contraction)
    T[y,x,d] = sum_c K[c,y,x,d]^2
    U[y,x]   = sum_c q[c,y,x]^2
    dot = box3x3(S); k2 = box3x3(T); q2 = box3x3(U)   (zero-padded box sums)
    sim = dot / (sqrt(q2)*sqrt(k2));  out = sim transposed to [B,D,H,W]

Sharding: 8 cores = (b in 2) x (y-quarter in 4). Each core computes 24 owned
rows + 1 halo row each side (26 rows, zero-padded at image edges).

Per-core device pipeline:
  Phase A (pixel-partition layout; partitions p=(y_sub,xb), free (c, xi, d)):
    DVE: broadcast-mult (q stride-0 over d) in c-halves, then pairwise-add
         tree to 8 c-slots (S path).
    ACT: K^2 in c-chunks; GPSIMD: pairwise add tree to 8 c-slots (T path).
    PE:  accumulating transposes of the 8 slots -> PSUM [d, pixels] finishes
         the channel reduction; ACT copies PSUM into the phase-B layout
         [(Yblk,d), (y_loc, x)] in SBUF.
    U path: ACT square + DVE reduce, bridged via a small DRAM round trip
    (re-read with a stride-0 AP to replicate U across the 32 d-partitions).
  Phase B ([(Yblk,d), (y 8, x 160)] tiles): separable 3x3 box via free-dim
    shifted adds (DVE for S, GPSIMD for T/U), rsqrt with one Newton step,
    multiply, store owned rows.
"""

import os
import numpy as np

B, C, H, W, D = 2, 32, 96, 160, 32
NCORES = 8
ROWS = 26  # 1 halo + 24 owned + 1 halo
YQ = H // 4  # 24 owned rows per core

# Phase-A blocks: (P, NY, NXB, XI, r0); p = y_sub*NXB + xb, free (C, XI, D)
BLOCKS = [
    (128, 8, 16, 10, 0),
    (128, 8, 16, 10, 8),
    (128, 8, 16, 10, 16),
    (64, 2, 32, 5, 24),
]
NYB = 4  # phase-B y-blocks, each covers rows [6j, 6j+8), owns y_loc 1..6

_CACHE = {}
LAST_EXEC_NS = None


def _build_program(reps=1):
    import concourse.bass as bass
    import concourse.bacc as bacc
    import concourse.mybir as mybir
    import concourse.tile as tile
    from contextlib import ExitStack

    f32 = mybir.dt.float32
    AF = mybir.ActivationFunctionType
    OP = mybir.AluOpType
    AX = mybir.AxisListType

    nc = bacc.Bacc("TRN2", target_bir_lowering=False, debug=False)

    kq_in, q_in = [], []
    for g, (P, NY, NXB, XI, r0) in enumerate(BLOCKS):
        kq_in.append(nc.dram_tensor(
            f"kq{g}", [P, C * XI * D], f32, kind="ExternalInput"))
        q_in.append(nc.dram_tensor(f"q{g}", [P, C * XI], f32, kind="ExternalInput"))
    id_in = nc.dram_tensor("ident", [128, 128], f32, kind="ExternalInput")
    o_out = nc.dram_tensor("o", [128, 6 * W], f32, kind="ExternalOutput")

    with tile.TileContext(nc) as tc, ExitStack() as ctx:
        dpool = ctx.enter_context(tc.tile_pool(name="dram", bufs=1, space="DRAM"))
        u_scr = dpool.tile([1, ROWS * W], f32)

        # phase-B accumulator tiles + identity live across both phases
        pbpool = ctx.enter_context(tc.tile_pool(name="pbacc", bufs=1))
        sd = pbpool.tile([128, 8 * W], f32)
        td = pbpool.tile([128, 8 * W], f32)
        idt = pbpool.tile([128, 128], f32)
        nc.scalar.dma_start(out=idt[:], in_=id_in[:])

        # timing variant: repeat the whole pipeline `reps` times
        for _rep in range(reps):
            pa = ExitStack()
            ppool = pa.enter_context(tc.tile_pool(name="prod", bufs=1))
            kqpool = pa.enter_context(tc.tile_pool(name="ksq", bufs=2))
            trpool = pa.enter_context(tc.tile_pool(name="tree", bufs=1))
            q2pool = pa.enter_context(tc.tile_pool(name="q2", bufs=1))
            upool = pa.enter_context(tc.tile_pool(name="u", bufs=4))
            pspool = pa.enter_context(tc.tile_pool(name="ps", bufs=1, space="PSUM"))

            CH = 8  # c-chunk size for ACT squares
            HC = C // 2

            def yb_overlaps(r0, NY):
                out = []
                for j in range(NYB):
                    lo, hi = max(r0, 6 * j), min(r0 + NY, 6 * j + 8)
                    if lo < hi:
                        out.append((j, lo - r0, hi - r0, lo - 6 * j))  # (Yb, y0, y1, yloc0)
                return out

            def box(z, eng, tag, pool, zb=None):
                zv = z[:].rearrange("p (y x) -> p y x", y=8, x=W)
                rxx = pool.tile([128, 8 * W], f32, tag="rxx_" + tag)
                rv = rxx[:].rearrange("p (y x) -> p y x", y=8, x=W)
                eng.tensor_add(rv[:, :, 0:W - 1], zv[:, :, 0:W - 1], zv[:, :, 1:W])
                eng.tensor_copy(rv[:, :, W - 1:W], zv[:, :, W - 1:W])
                eng.tensor_add(rv[:, :, 1:W], rv[:, :, 1:W], zv[:, :, 0:W - 1])
                if zb is None:
                    zb = pool.tile([128, 6 * W], f32, tag="zb_" + tag)
                zbv = zb[:].rearrange("p (y x) -> p y x", y=6, x=W)
                eng.tensor_add(zbv, rv[:, 0:6], rv[:, 1:7])
                eng.tensor_add(zbv, zbv, rv[:, 2:8])
                return zb

            # ---- U prologue: tiny q-only path for all blocks, feeds the
            # phase-B tail as early as possible ----
            qpool = pa.enter_context(tc.tile_pool(name="q", bufs=4))
            kpool = tc.alloc_tile_pool(name="k", bufs=2)
            q_tiles, u_tiles = [], []
            for g, (P, NY, NXB, XI, r0) in enumerate(BLOCKS):
                qt = qpool.tile([P, C * XI], f32, tag="qt")
                nc.sync.dma_start(out=qt[:], in_=q_in[g][:])
                qv = qt[:].rearrange("p (c x) -> p c x", c=C, x=XI)
                q_tiles.append(qv)
                q2 = q2pool.tile([P, C * XI], f32, tag="q2")
                nc.scalar.activation(q2[:], qv.squeeze(), AF.Square)
                u_t = upool.tile([P, XI], f32, tag="u")
                q2v = q2[:].rearrange("p (c x) -> p c x", c=C, x=XI)
                nc.vector.tensor_reduce(u_t[:], q2v.transpose([0, 2, 1]),
                                        axis=AX.X, op=OP.add)
                u_tiles.append(u_t)
            ud = pbpool.tile([128, 8 * W], f32, tag="ud")

            for g, (P, NY, NXB, XI, r0) in enumerate(BLOCKS):
                FD = XI * D
                HB = HC * FD  # half-K bytes in elems
                qv = q_tiles[g]
                kA = kpool.tile([P, HB], f32, tag="kA")
                nc.sync.dma_start(out=kA[:], in_=kq_in[g][:, :HB])
                kB = kpool.tile([P, HB], f32, tag="kB")
                nc.sync.dma_start(out=kB[:], in_=kq_in[g][:, HB:])
                udst = bass.AP(u_scr[:].tensor, u_scr[:].offset + r0 * W,
                               [[W, NY], [XI, NXB], [1, XI]])
                nc.sync.dma_start(out=udst, in_=u_tiles[g][:])
                if g == len(BLOCKS) - 1:
                    usrc = bass.AP(u_scr[:].tensor, u_scr[:].offset,
                                   [[6 * W, NYB], [0, D], [W, 8], [1, W]])  # d-replicate
                    nc.sync.dma_start(out=ud[:], in_=usrc)
                    ub = box(ud, nc.vector, "u", pbpool, zb=ud[:, 0:6 * W])
                kAv = kA[:].rearrange("p (c x d) -> p c x d", c=HC, x=XI, d=D)
                kBv = kB[:].rearrange("p (c x d) -> p c x d", c=HC, x=XI, d=D)

                # ---- S path: broadcast-mult (c-halves) + DVE tree to 8 slots ----
                sA1 = trpool.tile([P, (C // 2) * FD], f32, tag="sA1")
                sA1v = sA1[:].rearrange("p (c x d) -> p c x d", c=C // 2, x=XI, d=D)
                for h, kv in ((0, kAv), (1, kBv)):
                    prod = ppool.tile([P, HC * FD], f32, tag="prod")
                    pv = prod[:].rearrange("p (c x d) -> p c x d", c=HC, x=XI, d=D)
                    qb = qv[:, HC * h:HC * (h + 1)].unsqueeze(3).broadcast_to([P, HC, XI, D])
                    nc.vector.tensor_mul(pv, kv, qb)
                    nc.vector.tensor_add(
                        sA1v[:, h * HC // 2:(h + 1) * HC // 2],
                        pv[:, 0:HC:2], pv[:, 1:HC:2])
                sA2 = trpool.tile([P, (C // 4) * FD], f32, tag="sA2")
                sA2v = sA2[:].rearrange("p (c x d) -> p c x d", c=C // 4, x=XI, d=D)
                nc.vector.tensor_add(sA2v, sA1v[:, 0:C // 2:2], sA1v[:, 1:C // 2:2])

                # ---- T path: ACT squares (c-chunks) + GPSIMD tree to 8 slots ----
                tA1 = trpool.tile([P, (C // 2) * FD], f32, tag="tA1")
                tA1v = tA1[:].rearrange("p (c x d) -> p c x d", c=C // 2, x=XI, d=D)
                for ch in range(C // CH):
                    cph = HC // CH
                    kt, off = (kA, kB)[ch >= cph], (ch % cph) * CH * FD
                    ksq = kqpool.tile([P, CH * FD], f32, tag="ksq")
                    nc.scalar.activation(ksq[:], kt[:, off:off + CH * FD], AF.Square)
                    kqv = ksq[:].rearrange("p (c x d) -> p c x d", c=CH, x=XI, d=D)
                    nc.gpsimd.tensor_add(
                        tA1v[:, ch * CH // 2:(ch + 1) * CH // 2],
                        kqv[:, 0:CH:2], kqv[:, 1:CH:2])
                tA2 = trpool.tile([P, (C // 4) * FD], f32, tag="tA2")
                tA2v = tA2[:].rearrange("p (c x d) -> p c x d", c=C // 4, x=XI, d=D)
                nc.gpsimd.tensor_add(tA2v, tA1v[:, 0:C // 2:2], tA1v[:, 1:C // 2:2])

                # ---- PE bridge: accumulate the 8 slots while transposing ----
                NSLOT = C // 4
                for name, a2v, dst in (("s", sA2v, sd), ("t", tA2v, td)):
                    ps = pspool.tile([32, XI * P], f32, tag=f"ps{name}")
                    for xi in range(XI):
                        for slot in range(NSLOT):
                            nc.tensor.matmul(
                                ps[:, xi * P:(xi + 1) * P],
                                a2v[:, slot, xi, :],
                                idt[0:P, 0:P],
                                is_transpose=True,
                                start=(slot == 0), stop=(slot == NSLOT - 1))
                    for (j, y0, y1, yloc0) in yb_overlaps(r0, NY):
                        ny = y1 - y0
                        src = bass.AP(ps[:].tensor, ps[:].offset + y0 * NXB,
                                      [[XI * P, 32], [NXB, ny], [1, NXB], [P, XI]])
                        dstap = bass.AP(dst[:].tensor,
                                        dst[:].offset + (32 * j) * (8 * W) + yloc0 * W,
                                        [[8 * W, 32], [W, ny], [XI, NXB], [1, XI]])
                        nc.scalar.copy(dstap, src)

            # ================= Phase B =================
            kpool.release()  # only the big K pool must make room for phase B
            bpool = tc.alloc_tile_pool(name="pb", bufs=1)

            sb = box(sd, nc.gpsimd, "s", bpool)
            tb = box(td, nc.vector, "t", bpool)

            # ---- normalization: sim = Sb * rsqrt(Tb*Ub) with one Newton step ----
            t2 = bpool.tile([128, 6 * W], f32)
            nc.vector.tensor_mul(t2[:], tb[:], ub[:])
            den = bpool.tile([128, 6 * W], f32)
            nc.scalar.activation(den[:], t2[:], AF.Sqrt)
            y0 = bpool.tile([128, 6 * W], f32)
            nc.vector.reciprocal(y0[:], den[:])
            z = bpool.tile([128, 6 * W], f32)
            nc.vector.tensor_mul(z[:], sb[:], y0[:])
            w = bpool.tile([128, 6 * W], f32)
            nc.vector.tensor_mul(w[:], y0[:], y0[:])
            nc.vector.tensor_mul(w[:], w[:], t2[:])
            nc.vector.tensor_scalar(w[:], w[:], -0.5, 1.5, OP.mult, OP.add)
            sim = bpool.tile([128, 6 * W], f32)
            nc.vector.tensor_mul(sim[:], z[:], w[:])
            nc.sync.dma_start(out=o_out[:], in_=sim[:])
            bpool.release()
            pa.close()

    nc.compile()
    return nc


def _pack_core(qc, kc):
    """qc [C, ROWS, W], kc [C, ROWS, W, D] (halo rows included/zeroed) -> blobs."""
    blobs = {"ident": np.eye(128, dtype=np.float32)}
    for g, (P, NY, NXB, XI, r0) in enumerate(BLOCKS):
        kb = kc[:, r0:r0 + NY].reshape(C, NY, NXB, XI, D)
        blobs[f"kq{g}"] = np.ascontiguousarray(
            kb.transpose(1, 2, 0, 3, 4).reshape(P, C * XI * D))
        qb = qc[:, r0:r0 + NY].reshape(C, NY, NXB, XI)
        blobs[f"q{g}"] = np.ascontiguousarray(
            qb.transpose(1, 2, 0, 3).reshape(P, C * XI))
    return blobs


def make_in_maps(q, warped_feat):
    q = np.asarray(q, dtype=np.float32)
    k = np.asarray(warped_feat, dtype=np.float32)
    qp = np.zeros((B, C, H + 2, W), np.float32)
    kp = np.zeros((B, C, H + 2, W, D), np.float32)
    qp[:, :, 1:H + 1] = q
    kp[:, :, 1:H + 1] = k
    in_maps = []
    for core in range(NCORES):
        b, yq = divmod(core, 4)
        r = yq * YQ
        in_maps.append(_pack_core(qp[b, :, r:r + ROWS], kp[b, :, r:r + ROWS]))
    return in_maps


def unpack_out(results):
    out = np.empty((B, D, H, W), np.float32)
    for core in range(NCORES):
        b, yq = divmod(core, 4)
        blob = results[core]["o"].reshape(NYB, D, 6, W)
        for j in range(NYB):
            out[b, :, yq * YQ + 6 * j: yq * YQ + 6 * (j + 1), :] = blob[j]
    return out


def kernel(q, warped_feat):
    global LAST_EXEC_NS
    from concourse.bass_utils import run_bass_kernel_spmd

    if "nc" not in _CACHE:
        _CACHE["nc"] = _build_program()
    nc = _CACHE["nc"]
    in_maps = make_in_maps(q, warped_feat)
    trace = bool(int(os.environ.get("KERNEL_TRACE", "0")))
    res = run_bass_kernel_spmd(nc, in_maps, list(range(NCORES)), trace=trace)
    LAST_EXEC_NS = res.exec_time_ns
    return unpack_out(res.results)



# revision 32
# speedup vs baseline: 2.4032x; 2.4032x over previous
"""Trainium2 Bass kernel for DAV-block patch cosine similarity (v3, D-sharded).

Math (equivalent to the reference's 9-shift loop):
    S[y,x,d] = sum_c q[c,y,x]*K[c,y,x,d];  T = sum_c K^2;  U = sum_c q^2
    dot = box3x3(S); k2 = box3x3(T); q2 = box3x3(U)   (zero-padded box sums)
    sim = dot / (sqrt(q2)*sqrt(k2));  out = sim transposed to [B,D,H,W]

Sharding: 8 cores = (batch b in 2) x (d-slice j in 4); each core owns
D_loc=8 depth candidates over the FULL image (no spatial halo).

Per-core layout: partitions p = (y_loc 4, c 32); free = (d 8, x 160) = 1280.
The 96+2-row zero-padded image is packed as 25 groups of 4 rows (slab rows
0..99, rows 0 and 97..99 zero).

Pipeline (single pass, stream-paced):
  DMA kq group-chunks -> DVE products P_g = bf16(q*K) (q broadcast over d
  via a stride-0 *leading* free dim, keeping the packed last dim so the DVE
  2x 16-bit mode applies) and squares K2_g (DVE low d-rows / ACT rest).
  PE: per group, 3+3+1 matmuls with a ones-pattern stationary that fuses the
  c-reduction AND the vertical box sum: psS[96,1280] / psT / psU accumulate
  over all 25 groups in PSUM (out rows = 96 image rows; 3+3+1 banks).
  Post: horizontal box directly from PSUM (fp32), then
  sim = boxS * rsqrt(boxT) * rsqrt(boxU), fp32 out, per d-half for tail
  pipelining. Host reassembles [B,D,H,W].
"""

import os
import numpy as np
import ml_dtypes

B, C, H, W, D = 2, 32, 96, 160, 32
NCORES = 8
DL = D // 4          # 8 depth candidates per core; cores = (b, j)
NG = 25              # groups of 4 slab rows (slab = 100 rows, 4 zero rows)
FD = DL * W          # 1280 free elems per group
NPB = ml_dtypes.bfloat16

# d-column split of each group's K^2: [0:K2A) on DVE (mult), [K2A:FD) on ACT
K2A = 3 * W
# groups per kq DMA chunk
GCHUNK = 1

_CACHE = {}
LAST_EXEC_NS = None


def _build_program():
    import concourse.bass as bass
    import concourse.bacc as bacc
    import concourse.mybir as mybir
    import concourse.tile as tile
    from contextlib import ExitStack

    f32 = mybir.dt.float32
    bf16 = mybir.dt.bfloat16
    AF = mybir.ActivationFunctionType

    nc = bacc.Bacc("TRN2", target_bir_lowering=False, debug=False)

    kq_in = nc.dram_tensor("kq", [128, NG * FD], bf16, kind="ExternalInput")
    q_in = nc.dram_tensor("q", [128, NG * W], bf16, kind="ExternalInput")
    st_in = nc.dram_tensor("stat", [128, 192], bf16, kind="ExternalInput")
    o_out = nc.dram_tensor("o", [H, FD], bf16, kind="ExternalOutput")

    nchunk = (NG + GCHUNK - 1) // GCHUNK

    with tile.TileContext(nc) as tc, ExitStack() as ctx:
        cpool = ctx.enter_context(tc.tile_pool(name="const", bufs=1))
        kpool = ctx.enter_context(tc.tile_pool(name="k", bufs=5))
        ppool = ctx.enter_context(tc.tile_pool(name="p", bufs=8))
        k2pool = ctx.enter_context(tc.tile_pool(name="k2", bufs=8))
        bpool = ctx.enter_context(tc.tile_pool(name="pb", bufs=1))
        psSp = ctx.enter_context(tc.tile_pool(name="psS", bufs=1, space="PSUM"))
        psTp = ctx.enter_context(tc.tile_pool(name="psT", bufs=1, space="PSUM"))
        psUp = ctx.enter_context(tc.tile_pool(name="psU", bufs=1, space="PSUM"))

        # fill-critical DMAs first: the first q groups + the banded stationary
        # gate the first products/matmuls; the rest of q follows chunk 0.
        # q^2 is derived on-device (cheaper than shipping a second blob).
        QPRE = 4 * W
        qt = cpool.tile([128, NG * W], bf16)
        nc.sync.dma_start(out=qt[:, 0:QPRE], in_=q_in[:, 0:QPRE])
        st = cpool.tile([128, 192], bf16)
        nc.sync.dma_start(out=st[:], in_=st_in[:])
        q2t = cpool.tile([128, NG * W], bf16)
        nc.vector.tensor_mul(q2t[:, 0:QPRE], qt[:, 0:QPRE], qt[:, 0:QPRE])

        psS = psSp.tile([H, FD], f32)
        psT = psTp.tile([H, FD], f32)
        psU = psUp.tile([H, W], f32)

        # n-chunks of the 1280-wide psum rows, each within a 2KB bank
        MM = [(0, 512), (512, 512), (1024, 256)]

        for ci in range(nchunk):
            g0 = ci * GCHUNK
            gn = min(GCHUNK, NG - g0)
            kt = kpool.tile([128, gn * FD], bf16, tag="kt")
            # skip the all-zero padding rows of the first/last group's DMA
            if g0 == 0:
                nc.gpsimd.memset(kt[0:32, 0:FD], 0.0)
                nc.sync.dma_start(out=kt[32:128, 0:FD],
                                  in_=kq_in[32:128, 0:FD])
            elif g0 == NG - 1:
                for p0 in (32, 64, 96):
                    nc.gpsimd.memset(kt[p0:p0 + 32, 0:FD], 0.0)
                nc.sync.dma_start(out=kt[0:32, 0:FD],
                                  in_=kq_in[0:32, g0 * FD:(g0 + 1) * FD])
            else:
                nc.sync.dma_start(out=kt[:],
                                  in_=kq_in[:, g0 * FD:(g0 + gn) * FD])
            if ci == 2:
                nc.sync.dma_start(out=qt[:, QPRE:], in_=q_in[:, QPRE:])
                nc.vector.tensor_mul(q2t[:, QPRE:], qt[:, QPRE:],
                                     qt[:, QPRE:])
            for gi in range(gn):
                g = g0 + gi
                kv = kt[:, gi * FD:(gi + 1) * FD]
                kd = kv.rearrange("p (d x) -> p d x", d=DL, x=W)
                # products, q broadcast over leading d (keeps 2x mode)
                pt = ppool.tile([128, FD], bf16, tag="pt")
                pv = pt[:].rearrange("p (d x) -> p d x", d=DL, x=W)
                qb = qt[:, g * W:(g + 1) * W].unsqueeze(1).broadcast_to(
                    [128, DL, W])
                nc.vector.tensor_mul(pv, kd, qb)
                # squares, split DVE / ACT by d-columns (first groups all-DVE
                # so the first T-matmuls don't wait on ACT table loads)
                k2t = k2pool.tile([128, FD], bf16, tag="k2t")
                a = FD if g < 2 else K2A
                nc.vector.tensor_mul(k2t[:, 0:a], kv[:, 0:a], kv[:, 0:a])
                if a < FD:
                    nc.scalar.activation(k2t[:, a:FD], kv[:, a:FD], AF.Square)
                # PE: c-reduction + vertical box, accumulated over groups
                # (all stationaries are shifted slices of one banded matrix)
                stg = st[:, 96 - 4 * g:192 - 4 * g]
                first, last = (g == 0), (g == NG - 1)
                for off, n in MM:
                    nc.tensor.matmul(psS[:, off:off + n], stg,
                                     pt[:, off:off + n],
                                     start=first, stop=last)
                for off, n in MM:
                    nc.tensor.matmul(psT[:, off:off + n], stg,
                                     k2t[:, off:off + n],
                                     start=first, stop=last)
                nc.tensor.matmul(psU[:], stg, q2t[:, g * W:(g + 1) * W],
                                 start=first, stop=last)

        # ---- U path: horizontal box + rsqrt (tiny, [H, W]) ----
        # (hardware allows at most one PSUM operand per TensorTensor, so
        # every box is: copy psum->sbuf, then two sbuf+psum shifted adds)
        u0 = bpool.tile([H, W], f32)
        nc.vector.tensor_copy(u0[:], psU[:])
        ub = bpool.tile([H, W], f32)
        nc.vector.tensor_add(ub[:, 0:W - 1], u0[:, 0:W - 1], psU[:, 1:W])
        nc.vector.tensor_copy(ub[:, W - 1:W], u0[:, W - 1:W])
        nc.vector.tensor_add(ub[:, 1:W], ub[:, 1:W], psU[:, 0:W - 1])
        squ = bpool.tile([H, W], f32)
        nc.scalar.activation(squ[:], ub[:], AF.Sqrt)
        invu = bpool.tile([H, W], bf16)
        with nc.allow_low_precision(reason="bf16 norm chain, tol 2e-2"):
            nc.vector.reciprocal(invu[:], squ[:])

        # ---- S/T horizontal box from PSUM + normalization, per d-half ----
        DQ = DL // 2
        QW = DQ * W
        for h in range(2):
            c0 = h * QW
            sv = psS[:, c0:c0 + QW].rearrange("p (d x) -> p d x", d=DQ, x=W)
            tv = psT[:, c0:c0 + QW].rearrange("p (d x) -> p d x", d=DQ, x=W)

            # T's box gates the sqrt/recip chain -> ACT drain + DVE adds;
            # S's box only feeds the last multiply -> ACT drain + Pool adds
            # (GPSIMD cannot access PSUM, so Pool reads only the drained s0)
            s0 = bpool.tile([H, QW], bf16, tag=f"s0{h}")
            nc.scalar.activation(s0[:], psS[:, c0:c0 + QW], AF.Copy)
            s0v = s0[:].rearrange("p (d x) -> p d x", d=DQ, x=W)
            sb = bpool.tile([H, QW], bf16, tag=f"sb{h}")
            sbv = sb[:].rearrange("p (d x) -> p d x", d=DQ, x=W)
            nc.gpsimd.tensor_add(sbv[:, :, 0:W - 1], s0v[:, :, 0:W - 1],
                                 s0v[:, :, 1:W])
            nc.gpsimd.tensor_copy(sbv[:, :, W - 1:W], s0v[:, :, W - 1:W])
            nc.gpsimd.tensor_add(sbv[:, :, 1:W], sbv[:, :, 1:W],
                                 s0v[:, :, 0:W - 1])

            t0 = bpool.tile([H, QW], f32, tag=f"t0{h}")
            nc.scalar.activation(t0[:], psT[:, c0:c0 + QW], AF.Copy)
            t0v = t0[:].rearrange("p (d x) -> p d x", d=DQ, x=W)
            tb = bpool.tile([H, QW], f32, tag=f"tb{h}")
            tbv = tb[:].rearrange("p (d x) -> p d x", d=DQ, x=W)
            nc.vector.tensor_add(tbv[:, :, 0:W - 1], t0v[:, :, 0:W - 1],
                                 tv[:, :, 1:W])
            nc.vector.tensor_copy(tbv[:, :, W - 1:W], t0v[:, :, W - 1:W])
            nc.vector.tensor_add(tbv[:, :, 1:W], tbv[:, :, 1:W],
                                 tv[:, :, 0:W - 1])

            sqt = bpool.tile([H, QW], f32, tag=f"sqt{h}")
            nc.scalar.activation(sqt[:], tb[:], AF.Sqrt)
            invt = bpool.tile([H, QW], bf16, tag=f"invt{h}")
            with nc.allow_low_precision(reason="bf16 norm chain, tol 2e-2"):
                nc.vector.reciprocal(invt[:], sqt[:])
            itv = invt[:].rearrange("p (d x) -> p d x", d=DQ, x=W)
            iub = invu[:].unsqueeze(1).broadcast_to([H, DQ, W])
            nc.vector.tensor_mul(itv, itv, iub)
            sim = bpool.tile([H, QW], bf16, tag=f"sim{h}")
            nc.vector.tensor_mul(sim[:], sb[:], invt[:])
            nc.sync.dma_start(out=o_out[:, c0:c0 + QW], in_=sim[:])

    nc.compile()
    return nc


def _make_stat():
    """Banded stationary: B[(y_loc,c), j] = 1 iff j in {y_loc+94..y_loc+96};
    group g's stationary is the slice B[:, 96-4g : 192-4g] (device side)."""
    stat = np.zeros((4, 32, 192), np.float32)
    for yl in range(4):
        stat[yl, :, yl + 94:yl + 97] = 1.0
    return stat.reshape(128, 192).astype(NPB)


def make_in_maps(q, warped_feat):
    q = np.asarray(q, dtype=np.float32)
    k = np.asarray(warped_feat, dtype=np.float32)
    stat = _make_stat()
    in_maps = []
    for core in range(NCORES):
        b, j = divmod(core, 4)
        d0 = j * DL
        qsl = np.zeros((C, 4 * NG, W), np.float32)
        qsl[:, 1:H + 1] = q[b]
        ksl = np.zeros((C, 4 * NG, W, DL), np.float32)
        ksl[:, 1:H + 1] = k[b, :, :, :, d0:d0 + DL]
        # kq: [y4, c, g, d, x]
        kq = ksl.reshape(C, NG, 4, W, DL).transpose(2, 0, 1, 4, 3)
        qb = qsl.reshape(C, NG, 4, W).transpose(2, 0, 1, 3)
        in_maps.append({
            "kq": np.ascontiguousarray(kq.reshape(128, NG * FD)).astype(NPB),
            "q": np.ascontiguousarray(qb.reshape(128, NG * W)).astype(NPB),
            "stat": stat,
        })
    return in_maps


def unpack_out(results):
    out = np.empty((B, D, H, W), np.float32)
    for core in range(NCORES):
        b, j = divmod(core, 4)
        d0 = j * DL
        blob = results[core]["o"].astype(np.float32).reshape(H, DL, W)
        out[b, d0:d0 + DL] = blob.transpose(1, 0, 2)
    return out


def kernel(q, warped_feat):
    global LAST_EXEC_NS
    from concourse.bass_utils import run_bass_kernel_spmd

    if "nc" not in _CACHE:
        _CACHE["nc"] = _build_program()
    nc = _CACHE["nc"]
    in_maps = make_in_maps(q, warped_feat)
    trace = bool(int(os.environ.get("KERNEL_TRACE", "0")))
    res = run_bass_kernel_spmd(nc, in_maps, list(range(NCORES)), trace=trace)
    LAST_EXEC_NS = res.exec_time_ns
    return unpack_out(res.results)


# revision 36
# speedup vs baseline: 2.6469x; 1.1014x over previous
"""Trainium2 Bass kernel for DAV-block patch cosine similarity (v3, D-sharded).

Math (equivalent to the reference's 9-shift loop):
    S[y,x,d] = sum_c q[c,y,x]*K[c,y,x,d];  T = sum_c K^2;  U = sum_c q^2
    dot = box3x3(S); k2 = box3x3(T); q2 = box3x3(U)   (zero-padded box sums)
    sim = dot / (sqrt(q2)*sqrt(k2));  out = sim transposed to [B,D,H,W]

Sharding: 8 cores = (batch b in 2) x (d-slice j in 4); each core owns
D_loc=8 depth candidates over the FULL image (no spatial halo).

Per-core layout: partitions p = (y_loc 4, c 32); free = (d 8, x 160) = 1280.
The 96+2-row zero-padded image is packed as 25 groups of 4 rows (slab rows
0..99, rows 0 and 97..99 zero).

Pipeline (single pass, stream-paced):
  DMA kq group-chunks -> DVE products P_g = bf16(q*K) (q broadcast over d
  via a stride-0 *leading* free dim, keeping the packed last dim so the DVE
  2x 16-bit mode applies) and squares K2_g (DVE low d-rows / ACT rest).
  PE: per group, 3+3+1 matmuls with a ones-pattern stationary that fuses the
  c-reduction AND the vertical box sum: psS[96,1280] / psT / psU accumulate
  over all 25 groups in PSUM (out rows = 96 image rows; 3+3+1 banks).
  Post: horizontal box directly from PSUM (fp32), then
  sim = boxS * rsqrt(boxT) * rsqrt(boxU), fp32 out, per d-half for tail
  pipelining. Host reassembles [B,D,H,W].
"""

import os
import numpy as np
import ml_dtypes

B, C, H, W, D = 2, 32, 96, 160, 32
NCORES = 8
DL = D // 4          # 8 depth candidates per core; cores = (b, j)
NG = 25              # groups of 4 slab rows (slab = 100 rows, 4 zero rows)
FD = DL * W          # 1280 free elems per group
NPB = ml_dtypes.bfloat16

# d-column split of each group's K^2: [0:K2A) on DVE (mult), [K2A:FD) on ACT
K2A = 3 * W
# groups per kq DMA chunk
GCHUNK = 1

_CACHE = {}
LAST_EXEC_NS = None


def _build_program():
    import concourse.bass as bass
    import concourse.bacc as bacc
    import concourse.mybir as mybir
    import concourse.tile as tile
    from contextlib import ExitStack

    f32 = mybir.dt.float32
    bf16 = mybir.dt.bfloat16
    AF = mybir.ActivationFunctionType

    nc = bacc.Bacc("TRN2", target_bir_lowering=False, debug=False)

    kq_in = nc.dram_tensor("kq", [128, NG * FD], bf16, kind="ExternalInput")
    q_in = nc.dram_tensor("q", [128, NG * W], bf16, kind="ExternalInput")
    st_in = nc.dram_tensor("stat", [128, 192], bf16, kind="ExternalInput")
    o_out = nc.dram_tensor("o", [H, FD], bf16, kind="ExternalOutput")

    nchunk = (NG + GCHUNK - 1) // GCHUNK

    with tile.TileContext(nc) as tc, ExitStack() as ctx:
        cpool = ctx.enter_context(tc.tile_pool(name="const", bufs=1))
        kpool = ctx.enter_context(tc.tile_pool(name="k", bufs=5))
        ppool = ctx.enter_context(tc.tile_pool(name="p", bufs=8))
        k2pool = ctx.enter_context(tc.tile_pool(name="k2", bufs=8))
        bpool = ctx.enter_context(tc.tile_pool(name="pb", bufs=1))
        psSp = ctx.enter_context(tc.tile_pool(name="psS", bufs=1, space="PSUM"))
        psTp = ctx.enter_context(tc.tile_pool(name="psT", bufs=1, space="PSUM"))
        psUp = ctx.enter_context(tc.tile_pool(name="psU", bufs=1, space="PSUM"))

        # fill-critical DMAs first: the first q groups + the banded stationary
        # gate the first products/matmuls; the rest of q follows chunk 0.
        # q^2 is derived on-device (cheaper than shipping a second blob).
        QPRE = 4 * W
        qt = cpool.tile([128, NG * W], bf16)
        nc.sync.dma_start(out=qt[:, 0:QPRE], in_=q_in[:, 0:QPRE])
        st = cpool.tile([128, 192], bf16)
        nc.sync.dma_start(out=st[:], in_=st_in[:])
        q2t = cpool.tile([128, NG * W], bf16)
        nc.vector.tensor_mul(q2t[:, 0:QPRE], qt[:, 0:QPRE], qt[:, 0:QPRE])

        psS = psSp.tile([H, FD], f32)
        psT = psTp.tile([H, FD], f32)
        psU = psUp.tile([H, W], f32)

        # n-chunks of the 1280-wide psum rows, each within a 2KB bank
        MM = [(0, 512), (512, 512), (1024, 256)]

        for ci in range(nchunk):
            g0 = ci * GCHUNK
            gn = min(GCHUNK, NG - g0)
            kt = kpool.tile([128, gn * FD], bf16, tag="kt")
            # skip the all-zero padding rows of the first/last group's DMA
            if g0 == 0:
                nc.gpsimd.memset(kt[0:32, 0:FD], 0.0)
                nc.sync.dma_start(out=kt[32:128, 0:FD],
                                  in_=kq_in[32:128, 0:FD])
            elif g0 == NG - 1:
                for p0 in (32, 64, 96):
                    nc.gpsimd.memset(kt[p0:p0 + 32, 0:FD], 0.0)
                nc.sync.dma_start(out=kt[0:32, 0:FD],
                                  in_=kq_in[0:32, g0 * FD:(g0 + 1) * FD])
            else:
                nc.sync.dma_start(out=kt[:],
                                  in_=kq_in[:, g0 * FD:(g0 + gn) * FD])
            if ci == 2:
                nc.sync.dma_start(out=qt[:, QPRE:], in_=q_in[:, QPRE:])
                nc.vector.tensor_mul(q2t[:, QPRE:], qt[:, QPRE:],
                                     qt[:, QPRE:])
            for gi in range(gn):
                g = g0 + gi
                kv = kt[:, gi * FD:(gi + 1) * FD]
                kd = kv.rearrange("p (d x) -> p d x", d=DL, x=W)
                # products, q broadcast over leading d (keeps 2x mode)
                pt = ppool.tile([128, FD], bf16, tag="pt")
                pv = pt[:].rearrange("p (d x) -> p d x", d=DL, x=W)
                qb = qt[:, g * W:(g + 1) * W].unsqueeze(1).broadcast_to(
                    [128, DL, W])
                nc.vector.tensor_mul(pv, kd, qb)
                # squares, split DVE / ACT by d-columns (first groups all-DVE
                # so the first T-matmuls don't wait on ACT table loads)
                k2t = k2pool.tile([128, FD], bf16, tag="k2t")
                a = FD if g < 2 else K2A
                nc.vector.tensor_mul(k2t[:, 0:a], kv[:, 0:a], kv[:, 0:a])
                if a < FD:
                    nc.scalar.activation(k2t[:, a:FD], kv[:, a:FD], AF.Square)
                # PE: c-reduction + vertical box, accumulated over groups
                # (all stationaries are shifted slices of one banded matrix)
                stg = st[:, 96 - 4 * g:192 - 4 * g]
                first, last = (g == 0), (g == NG - 1)
                for off, n in MM:
                    nc.tensor.matmul(psS[:, off:off + n], stg,
                                     pt[:, off:off + n],
                                     start=first, stop=last)
                for off, n in MM:
                    nc.tensor.matmul(psT[:, off:off + n], stg,
                                     k2t[:, off:off + n],
                                     start=first, stop=last)
            if ci == 3:
                # U path: q2 is fully resident already -- run all U matmuls
                # now so invu is ready long before the tail
                for g in range(NG):
                    nc.tensor.matmul(psU[:],
                                     st[:, 96 - 4 * g:192 - 4 * g],
                                     q2t[:, g * W:(g + 1) * W],
                                     start=(g == 0), stop=(g == NG - 1))
            if ci == 8:
                # U horizontal box + rsqrt, emitted mid-stream (psU is long
                # done; also pre-loads the Sqrt table off the tail).  HW
                # allows only one PSUM operand per TensorTensor.
                u0 = bpool.tile([H, W], f32)
                nc.vector.tensor_copy(u0[:], psU[:])
                ub = bpool.tile([H, W], f32)
                nc.vector.tensor_add(ub[:, 0:W - 1], u0[:, 0:W - 1],
                                     psU[:, 1:W])
                nc.vector.tensor_copy(ub[:, W - 1:W], u0[:, W - 1:W])
                nc.vector.tensor_add(ub[:, 1:W], ub[:, 1:W], psU[:, 0:W - 1])
                squ = bpool.tile([H, W], f32)
                nc.scalar.activation(squ[:], ub[:], AF.Sqrt)
                invu = bpool.tile([H, W], bf16)
                with nc.allow_low_precision(reason="bf16 norm, tol 2e-2"):
                    nc.vector.reciprocal(invu[:], squ[:])

        # ---- S/T horizontal box from PSUM + normalization, per d-half ----
        DQ = DL // 2
        QW = DQ * W
        for h in range(2):
            c0 = h * QW
            sv = psS[:, c0:c0 + QW].rearrange("p (d x) -> p d x", d=DQ, x=W)
            tv = psT[:, c0:c0 + QW].rearrange("p (d x) -> p d x", d=DQ, x=W)

            # T's box gates the sqrt/recip chain -> ACT drain + DVE adds;
            # S's box only feeds the last multiply -> ACT drain + Pool adds
            # (GPSIMD cannot access PSUM, so Pool reads only the drained s0)
            s0 = bpool.tile([H, QW], bf16, tag=f"s0{h}")
            nc.scalar.activation(s0[:], psS[:, c0:c0 + QW], AF.Copy)
            s0v = s0[:].rearrange("p (d x) -> p d x", d=DQ, x=W)
            sb = bpool.tile([H, QW], bf16, tag=f"sb{h}")
            sbv = sb[:].rearrange("p (d x) -> p d x", d=DQ, x=W)
            nc.gpsimd.tensor_add(sbv[:, :, 0:W - 1], s0v[:, :, 0:W - 1],
                                 s0v[:, :, 1:W])
            nc.gpsimd.tensor_copy(sbv[:, :, W - 1:W], s0v[:, :, W - 1:W])
            nc.gpsimd.tensor_add(sbv[:, :, 1:W], sbv[:, :, 1:W],
                                 s0v[:, :, 0:W - 1])

            t0 = bpool.tile([H, QW], bf16, tag=f"t0{h}")
            nc.vector.tensor_copy(t0[:], psT[:, c0:c0 + QW])
            t0v = t0[:].rearrange("p (d x) -> p d x", d=DQ, x=W)
            tb = bpool.tile([H, QW], bf16, tag=f"tb{h}")
            tbv = tb[:].rearrange("p (d x) -> p d x", d=DQ, x=W)
            nc.vector.tensor_add(tbv[:, :, 0:W - 1], t0v[:, :, 0:W - 1],
                                 t0v[:, :, 1:W])
            nc.vector.tensor_copy(tbv[:, :, W - 1:W], t0v[:, :, W - 1:W])
            nc.vector.tensor_add(tbv[:, :, 1:W], tbv[:, :, 1:W],
                                 t0v[:, :, 0:W - 1])

            sqt = bpool.tile([H, QW], f32, tag=f"sqt{h}")
            nc.scalar.activation(sqt[:], tb[:], AF.Sqrt)
            invt = bpool.tile([H, QW], bf16, tag=f"invt{h}")
            with nc.allow_low_precision(reason="bf16 norm chain, tol 2e-2"):
                nc.vector.reciprocal(invt[:], sqt[:])
            itv = invt[:].rearrange("p (d x) -> p d x", d=DQ, x=W)
            iub = invu[:].unsqueeze(1).broadcast_to([H, DQ, W])
            nc.vector.tensor_mul(itv, itv, iub)
            sim = bpool.tile([H, QW], bf16, tag=f"sim{h}")
            nc.vector.tensor_mul(sim[:], sb[:], invt[:])
            nc.sync.dma_start(out=o_out[:, c0:c0 + QW], in_=sim[:])

    nc.compile()
    return nc


def _make_stat():
    """Banded stationary: B[(y_loc,c), j] = 1 iff j in {y_loc+94..y_loc+96};
    group g's stationary is the slice B[:, 96-4g : 192-4g] (device side)."""
    stat = np.zeros((4, 32, 192), np.float32)
    for yl in range(4):
        stat[yl, :, yl + 94:yl + 97] = 1.0
    return stat.reshape(128, 192).astype(NPB)


def make_in_maps(q, warped_feat):
    q = np.asarray(q, dtype=np.float32)
    k = np.asarray(warped_feat, dtype=np.float32)
    stat = _make_stat()
    in_maps = []
    for core in range(NCORES):
        b, j = divmod(core, 4)
        d0 = j * DL
        qsl = np.zeros((C, 4 * NG, W), np.float32)
        qsl[:, 1:H + 1] = q[b]
        ksl = np.zeros((C, 4 * NG, W, DL), np.float32)
        ksl[:, 1:H + 1] = k[b, :, :, :, d0:d0 + DL]
        # kq: [y4, c, g, d, x]
        kq = ksl.reshape(C, NG, 4, W, DL).transpose(2, 0, 1, 4, 3)
        qb = qsl.reshape(C, NG, 4, W).transpose(2, 0, 1, 3)
        in_maps.append({
            "kq": np.ascontiguousarray(kq.reshape(128, NG * FD)).astype(NPB),
            "q": np.ascontiguousarray(qb.reshape(128, NG * W)).astype(NPB),
            "stat": stat,
        })
    return in_maps


def unpack_out(results):
    out = np.empty((B, D, H, W), np.float32)
    for core in range(NCORES):
        b, j = divmod(core, 4)
        d0 = j * DL
        blob = results[core]["o"].astype(np.float32).reshape(H, DL, W)
        out[b, d0:d0 + DL] = blob.transpose(1, 0, 2)
    return out


def kernel(q, warped_feat):
    global LAST_EXEC_NS
    from concourse.bass_utils import run_bass_kernel_spmd

    if "nc" not in _CACHE:
        _CACHE["nc"] = _build_program()
    nc = _CACHE["nc"]
    in_maps = make_in_maps(q, warped_feat)
    trace = bool(int(os.environ.get("KERNEL_TRACE", "0")))
    res = run_bass_kernel_spmd(nc, in_maps, list(range(NCORES)), trace=trace)
    LAST_EXEC_NS = res.exec_time_ns
    return unpack_out(res.results)
